# revision 32
# baseline (speedup 1.0000x reference)
"""AMICO ADMM solver on 8 TRN2 NeuronCores.

min_x ||y - A x||^2 + lambda*|x|_1, x >= 0 via ADMM (100 iterations),
data-parallel over voxels (1024 voxels per core).

Shifted-variable restructuring (rho=1, kappa=lambda/rho), carrying
  vb := v - kappa   and   s := z - u + kappa = |vb|   (both fp16)
with v = x + u, x = D + W @ s, D = W@AtY - kappa*(W@1):
  psum = W @ s + D              # 8 fp16 matmuls + 4 fp16 identity injects
  vb'  = min(vb, 0) + psum      # ONE fused DVE scalar_tensor_tensor pass
                                #   (u/m never materializes)
  s'   = |vb'|                  # ScalarE Abs
Final output: x_100 = psum_100 directly.

Scheduling: work is quarter-sliced (r-block x column-half).  Each psum
quarter completes after 3 matmuls and its 512-wide stt+abs chain runs
while PE works the later quarters; the four D-injects are front-loaded
(no data deps) so PE has ready work at iteration start.  Every matmul
of the next iteration consumes both s' row-blocks, so the tail
quarter's chain is the critical cycle; 512-wide slices keep it short.
GpSimd is unusable (tensor ops ~15us each on HW); fp8 DoubleRow gave
no measured speedup over fp16 for the injects, so D stays fp16.
"""

import os

import numpy as np

M = 256
K = 256
N_VOX = 8192
N_CORES = 8
N_SHARD = N_VOX // N_CORES  # 1024
RHO = 1.0
LAMBDA_REG = 0.1
KAPPA = LAMBDA_REG / RHO
N_ITERS = 100

# Tuning knobs
D_MODE = os.environ.get("KERNEL_DMODE", "f16")  # 'f8c' | 'f16'
INJ_FRONT = bool(int(os.environ.get("KERNEL_INJ_FRONT", "1")))

LAST_RESULTS = None  # BassKernelResults of the most recent run (for test.py)


def _build_graph():
    import concourse.mybir as mybir
    from concourse import bacc
    from concourse.tile import TileContext

    f32 = mybir.dt.float32
    fp16 = mybir.dt.float16
    fp8 = mybir.dt.float8e4
    kap = float(KAPPA)
    alu = mybir.AluOpType

    nc = bacc.Bacc("TRN2", target_bir_lowering=False, debug=False)

    W16_p = nc.declare_dram_parameter("W16", [128, 512], fp16, isOutput=False)
    if D_MODE == "f8c":
        D8_p = nc.declare_dram_parameter("D8", [128, 4, 1024], fp8, isOutput=False)
        I2_p = nc.declare_dram_parameter("I2", [128, 2, 128], fp8, isOutput=False)
    else:
        D16_p = nc.declare_dram_parameter("D16", [128, 2048], fp16, isOutput=False)
        I16_p = nc.declare_dram_parameter("I16", [128, 128], fp16, isOutput=False)
    S1_p = nc.declare_dram_parameter("S1", [128, 2048], fp16, isOutput=False)
    VB1_p = nc.declare_dram_parameter("VB1", [128, 2048], fp16, isOutput=False)
    O_p = nc.declare_dram_parameter("out", [128, 2048], fp16, isOutput=True)

    absf = mybir.ActivationFunctionType.Abs
    dr = mybir.MatmulPerfMode.DoubleRow

    with TileContext(nc) as tc:
        with (
            tc.tile_pool(name="static", bufs=1) as statics,
            tc.tile_pool(name="spool", bufs=3) as spool,
            tc.tile_pool(name="vpool", bufs=3) as vpool,
            tc.tile_pool(name="mpool", bufs=3) as mpool,
        ):
            # Spread the input DMAs across engine DGE queues so they run in
            # parallel at startup (one queue would serialize ~650 ns each).
            W16_sb = statics.tile([128, 512], fp16, name="W16_sb")
            if D_MODE == "f8c":
                D8_sb = statics.tile([128, 4, 1024], fp8, name="D8_sb")
                nc.sync.dma_start(D8_sb[:, :, :], D8_p[:, :, :])
                I2_sb = statics.tile([128, 2, 128], fp8, name="I2_sb")
                nc.scalar.dma_start(I2_sb[:, :, :], I2_p[:, :, :])
            else:
                D16_sb = statics.tile([128, 2048], fp16, name="D16_sb")
                nc.sync.dma_start(D16_sb[:, 0:1024], D16_p[:, 0:1024])
                nc.gpsimd.dma_start(D16_sb[:, 1024:2048], D16_p[:, 1024:2048])
                I16_sb = statics.tile([128, 128], fp16, name="I16_sb")
                nc.scalar.dma_start(I16_sb[:, :], I16_p[:, :])
            nc.sync.dma_start(W16_sb[:, :], W16_p[:, :])
            out_sb = statics.tile([128, 2048], fp16, name="out_sb")

            # Warm the ScalarE activation table (Abs) outside the loop.
            warm_in = statics.tile([1, 8], fp16, name="warm_in")
            nc.vector.memset(warm_in[:, :], 0.25)
            warm_sb = statics.tile([1, 8], fp16, name="warm_sb")
            nc.scalar.activation(warm_sb[:, :], warm_in[:, :], absf)

            # Iteration 1 is folded into the host precompute (x_1 = W@AtY is
            # a byproduct of computing D): upload s_1, vb_1 and run 99
            # device iterations.
            s_cur = spool.tile([128, 2048], fp16, name="s_new", tag="s")
            nc.scalar.dma_start(s_cur[:, :], S1_p[:, :])
            vb_cur = vpool.tile([128, 2048], fp16, name="vb", tag="vb")
            nc.gpsimd.dma_start(vb_cur[:, :], VB1_p[:, :])

            # Junk matmuls whose only dependency is a cheap DVE memset: ramp
            # the PE clock out of its low p-state while input DMAs stream in.
            warm16 = statics.tile([128, 128], fp16, name="warm16")
            nc.vector.memset(warm16[:, :], 0.5)

            with tc.tile_pool(name="psum_loop", bufs=4, space="PSUM") as psl:
                ps_warm = psl.tile([128, 1024], f32, name="ps_x", tag="ps")
                for _ in range(16):
                    nc.tensor.matmul(
                        ps_warm[:, 0:64],
                        warm16[:, :],
                        warm16[:, 0:64],
                        start=True,
                        stop=True,
                        skip_group_check=True,
                    )
                for it in range(N_ITERS - 1):
                    last = it == N_ITERS - 2
                    ps = [None, None]
                    for r in (0, 1):
                        ps[r] = psl.tile([128, 1024], f32, name="ps_x", tag="ps")
                    if not last:
                        vb_new = vpool.tile([128, 2048], fp16, name="vb", tag="vb")
                        sn = spool.tile([128, 2048], fp16, name="s_new", tag="s")

                    # Quarters q = (r, ch).  Every matmul of the NEXT
                    # iteration consumes both s' row-blocks (kc0 reads b0,
                    # kc1 reads b1), so the last quarter's stt+abs chain
                    # gates the cycle.  Stagger the 12 matmuls so the
                    # kc1-consumers run ~2 us into the iteration, hiding the
                    # tail chain behind PE work.
                    QUARTERS = [(0, 0), (0, 1), (1, 0), (1, 1)]

                    def inj(q):
                        r, ch = QUARTERS[q]
                        cs = slice(ch * 512, ch * 512 + 512)
                        if D_MODE == "f8c":
                            nc.tensor.matmul(
                                ps[r][:, cs],
                                I2_sb[:, :, :],
                                D8_sb[:, 2 * r : 2 * r + 2, cs],
                                start=True,
                                stop=False,
                                perf_mode=dr,
                                skip_group_check=True,
                            )
                        else:
                            nc.tensor.matmul(
                                ps[r][:, cs],
                                I16_sb[:, :],
                                D16_sb[:, r * 1024 + ch * 512 : r * 1024 + ch * 512 + 512],
                                start=True,
                                stop=False,
                                skip_group_check=True,
                            )

                    def wmm(kc, q):
                        r, ch = QUARTERS[q]
                        cs = slice(ch * 512, ch * 512 + 512)
                        nc.tensor.matmul(
                            ps[r][:, cs],
                            W16_sb[:, kc * 256 + r * 128 : kc * 256 + r * 128 + 128],
                            s_cur[:, kc * 1024 + ch * 512 : kc * 1024 + ch * 512 + 512],
                            start=False,
                            stop=(kc == 1),
                            skip_group_check=True,
                        )

                    def stt(q):
                        r, ch = QUARTERS[q]
                        cs = slice(ch * 512, ch * 512 + 512)
                        qs = slice(r * 1024 + ch * 512, r * 1024 + ch * 512 + 512)
                        # vb' = min(vb, 0) + psum (fused 1x DVE pass).
                        nc.vector.scalar_tensor_tensor(
                            vb_new[:, qs],
                            vb_cur[:, qs],
                            0.0,
                            ps[r][:, cs],
                            alu.min,
                            alu.add,
                        )

                    # PE order: injects have no data dependencies; front-
                    # loading them gives PE ready work at iteration start to
                    # absorb chain jitter.  Then per quarter [kc0, kc1]; each
                    # quarter's stt+abs chain runs while PE works the later
                    # quarters.
                    if INJ_FRONT:
                        for q in range(4):
                            inj(q)
                    for q in range(4):
                        if not INJ_FRONT:
                            inj(q)
                        wmm(0, q)
                        wmm(1, q)
                        r, ch = QUARTERS[q]
                        qs = slice(r * 1024 + ch * 512, r * 1024 + ch * 512 + 512)
                        if last:
                            # Alternate Act/DVE so the copies run in parallel.
                            if q % 2 == 0:
                                nc.scalar.copy(out_sb[:, qs], ps[r][:, ch * 512 : ch * 512 + 512])
                            else:
                                nc.vector.tensor_copy(out_sb[:, qs], ps[r][:, ch * 512 : ch * 512 + 512])
                            dma_eng = nc.sync if q % 2 == 0 else nc.gpsimd
                            dma_eng.dma_start(O_p[:, qs], out_sb[:, qs])
                            continue
                        stt(q)
                        nc.scalar.activation(sn[:, qs], vb_new[:, qs], absf)
                    if last:
                        break
                    s_cur, vb_cur = sn, vb_new

    nc.compile()
    return nc


_GRAPH = None


def kernel(A: np.ndarray, data: np.ndarray) -> np.ndarray:
    global _GRAPH, LAST_RESULTS
    import ml_dtypes
    from concourse.bass_utils import run_bass_kernel_spmd

    F8 = ml_dtypes.float8_e4m3

    A = np.ascontiguousarray(np.asarray(A, dtype=np.float32))
    data = np.ascontiguousarray(np.asarray(data, dtype=np.float32))
    assert A.shape == (M, K) and data.shape == (N_VOX, M)

    A64 = A.astype(np.float64)
    AtA = A64.T @ A64
    W = np.linalg.inv(AtA + RHO * np.eye(K))
    w1 = KAPPA * (W @ np.ones(K))

    # W16[p, kc*256 + c] = W[kc*128+p, c]  (W symmetric)
    W_dev = (
        W.astype(np.float32).reshape(2, 128, K).transpose(1, 0, 2).reshape(128, 2 * K)
    )
    W16_dev = W_dev.astype(np.float16)

    if D_MODE == "f8c":
        # I2[k, j, p] = (p == k) for j in {0,1}
        i2 = np.zeros((128, 2, 128), dtype=F8)
        eye = np.eye(128, dtype=np.float32).astype(F8)
        i2[:, 0, :] = eye
        i2[:, 1, :] = eye
    else:
        i16 = np.eye(128, dtype=np.float16)

    def to_dev16(X):
        # [256, 1024] -> [128, 2048] with cols = r*1024 + n
        return np.ascontiguousarray(
            X.astype(np.float16)
            .reshape(2, 128, N_SHARD)
            .transpose(1, 0, 2)
            .reshape(128, 2 * N_SHARD)
        )

    in_maps = []
    for i in range(N_CORES):
        shard = data[i * N_SHARD : (i + 1) * N_SHARD]  # [1024, 256]
        AtY = A64.T @ shard.astype(np.float64).T  # [256, 1024]
        WAtY = W @ AtY  # [256, 1024] f64 (= x_1)
        D = WAtY - w1[:, None]  # [256, 1024] f64
        # Iteration 1 on host: x_1 = W@AtY, vb_1 = x_1 - kappa, s_1 = |vb_1|.
        vb1 = WAtY - KAPPA
        extra = {"S1": to_dev16(np.abs(vb1)), "VB1": to_dev16(vb1)}
        if D_MODE == "f8c":
            Dr = D.reshape(2, 128, N_SHARD)  # [r, p, n]
            Da = Dr.astype(F8)
            Db = (Dr - Da.astype(np.float64)).astype(F8)
            D8 = np.empty((128, 4, N_SHARD), dtype=F8)
            for r in (0, 1):
                D8[:, 2 * r + 0, :] = Da[r]
                D8[:, 2 * r + 1, :] = Db[r]
            in_maps.append(
                {
                    "W16": W16_dev,
                    "D8": np.ascontiguousarray(D8),
                    "I2": np.ascontiguousarray(i2),
                    **extra,
                }
            )
        else:
            in_maps.append(
                {
                    "W16": W16_dev,
                    "D16": to_dev16(D),
                    "I16": i16,
                    **extra,
                }
            )
    if _GRAPH is None:
        _GRAPH = _build_graph()

    trace = bool(int(os.environ.get("KERNEL_TRACE", "0")))
    res = run_bass_kernel_spmd(
        _GRAPH, in_maps, core_ids=list(range(N_CORES)), trace=trace
    )
    LAST_RESULTS = res

    out = np.empty((N_VOX, K), dtype=np.float32)
    for i in range(N_CORES):
        o = np.asarray(res.results[i]["out"], dtype=np.float32)  # [128, 2048] fp16
        X = o.reshape(128, 2, N_SHARD).transpose(1, 0, 2).reshape(K, N_SHARD)
        out[i * N_SHARD : (i + 1) * N_SHARD] = X.T
    return out


# revision 35
# speedup vs baseline: 1.0037x; 1.0037x over previous
"""AMICO ADMM solver on 8 TRN2 NeuronCores.

min_x ||y - A x||^2 + lambda*|x|_1, x >= 0 via ADMM (100 iterations),
data-parallel over voxels (1024 voxels per core).

Shifted-variable restructuring (rho=1, kappa=lambda/rho), carrying
  vb := v - kappa   and   s := z - u + kappa = |vb|   (both fp16)
with v = x + u, x = D + W @ s, D = W@AtY - kappa*(W@1):
  psum = W @ s + D              # 8 fp16 matmuls + 4 fp16 identity injects
  vb'  = min(vb, 0) + psum      # ONE fused DVE scalar_tensor_tensor pass
                                #   (u/m never materializes)
  s'   = |vb'|                  # ScalarE Abs
Final output: x_100 = psum_100 directly.

Scheduling: work is quarter-sliced (r-block x column-half).  Each psum
quarter completes after 3 matmuls and its 512-wide stt+abs chain runs
while PE works the later quarters; the four D-injects are front-loaded
(no data deps) so PE has ready work at iteration start.  Every matmul
of the next iteration consumes both s' row-blocks, so the tail
quarter's chain is the critical cycle; 512-wide slices keep it short.
GpSimd is unusable (tensor ops ~15us each on HW); fp8 DoubleRow gave
no measured speedup over fp16 for the injects, so D stays fp16.
"""

import os

import numpy as np

M = 256
K = 256
N_VOX = 8192
N_CORES = 8
N_SHARD = N_VOX // N_CORES  # 1024
RHO = 1.0
LAMBDA_REG = 0.1
KAPPA = LAMBDA_REG / RHO
N_ITERS = 100

# Tuning knobs
D_MODE = os.environ.get("KERNEL_DMODE", "f16")  # 'f8c' | 'f16'
INJ_FRONT = bool(int(os.environ.get("KERNEL_INJ_FRONT", "1")))

LAST_RESULTS = None  # BassKernelResults of the most recent run (for test.py)


def _build_graph():
    import concourse.mybir as mybir
    from concourse import bacc
    from concourse.tile import TileContext

    f32 = mybir.dt.float32
    fp16 = mybir.dt.float16
    fp8 = mybir.dt.float8e4
    kap = float(KAPPA)
    alu = mybir.AluOpType

    nc = bacc.Bacc("TRN2", target_bir_lowering=False, debug=False)

    W16_p = nc.declare_dram_parameter("W16", [128, 512], fp16, isOutput=False)
    if D_MODE == "f8c":
        D8_p = nc.declare_dram_parameter("D8", [128, 4, 1024], fp8, isOutput=False)
        I2_p = nc.declare_dram_parameter("I2", [128, 2, 128], fp8, isOutput=False)
    else:
        D16_p = nc.declare_dram_parameter("D16", [128, 2048], fp16, isOutput=False)
        I16_p = nc.declare_dram_parameter("I16", [128, 128], fp16, isOutput=False)
    S1_p = nc.declare_dram_parameter("S1", [128, 2048], fp16, isOutput=False)
    VB1_p = nc.declare_dram_parameter("VB1", [128, 2048], fp16, isOutput=False)
    O_p = nc.declare_dram_parameter("out", [128, 2048], fp16, isOutput=True)

    absf = mybir.ActivationFunctionType.Abs
    dr = mybir.MatmulPerfMode.DoubleRow

    with TileContext(nc) as tc:
        with (
            tc.tile_pool(name="static", bufs=1) as statics,
            tc.tile_pool(name="spool", bufs=3) as spool,
            tc.tile_pool(name="vpool", bufs=3) as vpool,
            tc.tile_pool(name="mpool", bufs=3) as mpool,
        ):
            # Spread the input DMAs across engine DGE queues so they run in
            # parallel at startup (one queue would serialize ~650 ns each).
            W16_sb = statics.tile([128, 512], fp16, name="W16_sb")
            if D_MODE == "f8c":
                D8_sb = statics.tile([128, 4, 1024], fp8, name="D8_sb")
                nc.sync.dma_start(D8_sb[:, :, :], D8_p[:, :, :])
                I2_sb = statics.tile([128, 2, 128], fp8, name="I2_sb")
                nc.scalar.dma_start(I2_sb[:, :, :], I2_p[:, :, :])
            else:
                D16_sb = statics.tile([128, 2048], fp16, name="D16_sb")
                nc.sync.dma_start(D16_sb[:, 0:1024], D16_p[:, 0:1024])
                nc.gpsimd.dma_start(D16_sb[:, 1024:2048], D16_p[:, 1024:2048])
                I16_sb = statics.tile([128, 128], fp16, name="I16_sb")
                nc.scalar.dma_start(I16_sb[:, :], I16_p[:, :])
            nc.sync.dma_start(W16_sb[:, :], W16_p[:, :])
            out_sb = statics.tile([128, 2048], fp16, name="out_sb")

            # Warm the ScalarE activation table (Abs) outside the loop.
            warm_in = statics.tile([1, 8], fp16, name="warm_in")
            nc.vector.memset(warm_in[:, :], 0.25)
            warm_sb = statics.tile([1, 8], fp16, name="warm_sb")
            nc.scalar.activation(warm_sb[:, :], warm_in[:, :], absf)

            # Iteration 1 is folded into the host precompute (x_1 = W@AtY is
            # a byproduct of computing D): upload s_1, vb_1 and run 99
            # device iterations.
            s_cur = spool.tile([128, 2048], fp16, name="s_new", tag="s")
            nc.scalar.dma_start(s_cur[:, :], S1_p[:, :])
            vb_cur = vpool.tile([128, 2048], fp16, name="vb", tag="vb")
            nc.gpsimd.dma_start(vb_cur[:, :], VB1_p[:, :])

            # Junk matmuls whose only dependency is a cheap DVE memset: ramp
            # the PE clock out of its low p-state while input DMAs stream in.
            warm16 = statics.tile([128, 128], fp16, name="warm16")
            nc.vector.memset(warm16[:, :], 0.5)

            with tc.tile_pool(name="psum_loop", bufs=4, space="PSUM") as psl:
                ps_warm = psl.tile([128, 1024], f32, name="ps_x", tag="ps")
                for _ in range(24):
                    nc.tensor.matmul(
                        ps_warm[:, 0:64],
                        warm16[:, :],
                        warm16[:, 0:64],
                        start=True,
                        stop=True,
                        skip_group_check=True,
                    )
                for it in range(N_ITERS - 1):
                    last = it == N_ITERS - 2
                    ps = [None, None]
                    for r in (0, 1):
                        ps[r] = psl.tile([128, 1024], f32, name="ps_x", tag="ps")
                    if not last:
                        vb_new = vpool.tile([128, 2048], fp16, name="vb", tag="vb")
                        sn = spool.tile([128, 2048], fp16, name="s_new", tag="s")

                    # Quarters q = (r, ch).  Every matmul of the NEXT
                    # iteration consumes both s' row-blocks (kc0 reads b0,
                    # kc1 reads b1), so the last quarter's stt+abs chain
                    # gates the cycle.  Stagger the 12 matmuls so the
                    # kc1-consumers run ~2 us into the iteration, hiding the
                    # tail chain behind PE work.
                    QUARTERS = [(0, 0), (0, 1), (1, 0), (1, 1)]

                    def inj(q):
                        r, ch = QUARTERS[q]
                        cs = slice(ch * 512, ch * 512 + 512)
                        if D_MODE == "f8c":
                            nc.tensor.matmul(
                                ps[r][:, cs],
                                I2_sb[:, :, :],
                                D8_sb[:, 2 * r : 2 * r + 2, cs],
                                start=True,
                                stop=False,
                                perf_mode=dr,
                                skip_group_check=True,
                            )
                        else:
                            nc.tensor.matmul(
                                ps[r][:, cs],
                                I16_sb[:, :],
                                D16_sb[:, r * 1024 + ch * 512 : r * 1024 + ch * 512 + 512],
                                start=True,
                                stop=False,
                                skip_group_check=True,
                            )

                    def wmm(kc, q):
                        r, ch = QUARTERS[q]
                        cs = slice(ch * 512, ch * 512 + 512)
                        nc.tensor.matmul(
                            ps[r][:, cs],
                            W16_sb[:, kc * 256 + r * 128 : kc * 256 + r * 128 + 128],
                            s_cur[:, kc * 1024 + ch * 512 : kc * 1024 + ch * 512 + 512],
                            start=False,
                            stop=(kc == 1),
                            skip_group_check=True,
                        )

                    def stt(q):
                        r, ch = QUARTERS[q]
                        cs = slice(ch * 512, ch * 512 + 512)
                        qs = slice(r * 1024 + ch * 512, r * 1024 + ch * 512 + 512)
                        # vb' = min(vb, 0) + psum (fused 1x DVE pass).
                        nc.vector.scalar_tensor_tensor(
                            vb_new[:, qs],
                            vb_cur[:, qs],
                            0.0,
                            ps[r][:, cs],
                            alu.min,
                            alu.add,
                        )

                    # PE order: injects have no data dependencies; front-
                    # loading them gives PE ready work at iteration start to
                    # absorb chain jitter.  Then per quarter [kc0, kc1]; each
                    # quarter's stt+abs chain runs while PE works the later
                    # quarters.
                    if INJ_FRONT:
                        for q in range(4):
                            inj(q)
                    for q in range(4):
                        if not INJ_FRONT:
                            inj(q)
                        wmm(0, q)
                        wmm(1, q)
                        r, ch = QUARTERS[q]
                        qs = slice(r * 1024 + ch * 512, r * 1024 + ch * 512 + 512)
                        if last:
                            # Alternate Act/DVE so the copies run in parallel.
                            if q % 2 == 0:
                                nc.scalar.copy(out_sb[:, qs], ps[r][:, ch * 512 : ch * 512 + 512])
                            else:
                                nc.vector.tensor_copy(out_sb[:, qs], ps[r][:, ch * 512 : ch * 512 + 512])
                            dma_eng = nc.sync if q % 2 == 0 else nc.gpsimd
                            dma_eng.dma_start(O_p[:, qs], out_sb[:, qs])
                            continue
                        stt(q)
                        if it == 0 and q >= 2:
                            # Pipeline-fill accelerator: iteration 0 has no
                            # previous iteration to hide behind, so its tail
                            # abs quarters queue ~3us deep on Act.  Run them
                            # on DVE instead: fp16 |x| = clear the sign bit.
                            u16 = mybir.dt.uint16
                            nc.vector.tensor_scalar(
                                sn[:, qs].bitcast(u16),
                                vb_new[:, qs].bitcast(u16),
                                0x7FFF,
                                None,
                                alu.bitwise_and,
                            )
                        else:
                            nc.scalar.activation(sn[:, qs], vb_new[:, qs], absf)
                    if last:
                        break
                    s_cur, vb_cur = sn, vb_new

    nc.compile()
    return nc


_GRAPH = None


def kernel(A: np.ndarray, data: np.ndarray) -> np.ndarray:
    global _GRAPH, LAST_RESULTS
    import ml_dtypes
    from concourse.bass_utils import run_bass_kernel_spmd

    F8 = ml_dtypes.float8_e4m3

    A = np.ascontiguousarray(np.asarray(A, dtype=np.float32))
    data = np.ascontiguousarray(np.asarray(data, dtype=np.float32))
    assert A.shape == (M, K) and data.shape == (N_VOX, M)

    A64 = A.astype(np.float64)
    AtA = A64.T @ A64
    W = np.linalg.inv(AtA + RHO * np.eye(K))
    w1 = KAPPA * (W @ np.ones(K))

    # W16[p, kc*256 + c] = W[kc*128+p, c]  (W symmetric)
    W_dev = (
        W.astype(np.float32).reshape(2, 128, K).transpose(1, 0, 2).reshape(128, 2 * K)
    )
    W16_dev = W_dev.astype(np.float16)

    if D_MODE == "f8c":
        # I2[k, j, p] = (p == k) for j in {0,1}
        i2 = np.zeros((128, 2, 128), dtype=F8)
        eye = np.eye(128, dtype=np.float32).astype(F8)
        i2[:, 0, :] = eye
        i2[:, 1, :] = eye
    else:
        i16 = np.eye(128, dtype=np.float16)

    def to_dev16(X):
        # [256, 1024] -> [128, 2048] with cols = r*1024 + n
        return np.ascontiguousarray(
            X.astype(np.float16)
            .reshape(2, 128, N_SHARD)
            .transpose(1, 0, 2)
            .reshape(128, 2 * N_SHARD)
        )

    in_maps = []
    for i in range(N_CORES):
        shard = data[i * N_SHARD : (i + 1) * N_SHARD]  # [1024, 256]
        AtY = A64.T @ shard.astype(np.float64).T  # [256, 1024]
        WAtY = W @ AtY  # [256, 1024] f64 (= x_1)
        D = WAtY - w1[:, None]  # [256, 1024] f64
        # Iteration 1 on host: x_1 = W@AtY, vb_1 = x_1 - kappa, s_1 = |vb_1|.
        vb1 = WAtY - KAPPA
        extra = {"S1": to_dev16(np.abs(vb1)), "VB1": to_dev16(vb1)}
        if D_MODE == "f8c":
            Dr = D.reshape(2, 128, N_SHARD)  # [r, p, n]
            Da = Dr.astype(F8)
            Db = (Dr - Da.astype(np.float64)).astype(F8)
            D8 = np.empty((128, 4, N_SHARD), dtype=F8)
            for r in (0, 1):
                D8[:, 2 * r + 0, :] = Da[r]
                D8[:, 2 * r + 1, :] = Db[r]
            in_maps.append(
                {
                    "W16": W16_dev,
                    "D8": np.ascontiguousarray(D8),
                    "I2": np.ascontiguousarray(i2),
                    **extra,
                }
            )
        else:
            in_maps.append(
                {
                    "W16": W16_dev,
                    "D16": to_dev16(D),
                    "I16": i16,
                    **extra,
                }
            )
    if _GRAPH is None:
        _GRAPH = _build_graph()

    trace = bool(int(os.environ.get("KERNEL_TRACE", "0")))
    res = run_bass_kernel_spmd(
        _GRAPH, in_maps, core_ids=list(range(N_CORES)), trace=trace
    )
    LAST_RESULTS = res

    out = np.empty((N_VOX, K), dtype=np.float32)
    for i in range(N_CORES):
        o = np.asarray(res.results[i]["out"], dtype=np.float32)  # [128, 2048] fp16
        X = o.reshape(128, 2, N_SHARD).transpose(1, 0, 2).reshape(K, N_SHARD)
        out[i * N_SHARD : (i + 1) * N_SHARD] = X.T
    return out
